# revision 30
# baseline (speedup 1.0000x reference)
"""MQA attention kernel for Trainium2, 8-core SPMD.

Problem: Q [2, 8, 2048, 64] fp32, K/V [2, 1, 2048, 64] fp32 (shared head).
out[b,h,q,:] = softmax(Q[b,h,q,:] @ K[b,0]^T / 8) @ V[b,0].

Sharding: 16 (b,h) pairs over 8 cores -> core c handles b = c//4,
heads 2*(c%4), 2*(c%4)+1 (both heads share one K/V slice).

Per-core design (matmuls fp16, accumulation fp32 in PSUM).  The kernel is
bound by the scalar engine's exp throughput (1 elem/cycle/lane @ 1.2GHz:
~1.0-1.1us per [128, 2x512] score tile, 64 tiles), so everything else is
arranged to keep the ACT stream saturated:

  - Staging: the time-critical K and pass-0 Q arrive over the sync HWDGE
    queue as fp32 with DVE casts; V and the remaining Q use gpsimd cast
    DMAs in parallel.  PE transposes (fp16) build K^T/Q^T incrementally --
    the lead-in stages only what pass 0's first iterations need, the rest
    is staged just-in-time inside the main loop, 2 tiles ahead of the MM1
    that first reads it.
  - Main loop: one flat stream over i = (pass, kt); MM1 for i+2 is emitted
    ahead of MM2(i) (scores triple-buffered in PSUM) so the ACT->MM2
    dependence never idles the PE; the two heads' MM1s run concurrently on
    different PE row-quadrants (contract=64).
  - ACT: exp of the [128, 2, 512] fp32 score tile with the 1/8 scale
    folded into the activation's free affine, writing fp16 P^T directly.
  - MM2: out^T[h][65, 512] += V_aug[kt]^T @ P^T[kt, h]; V_aug carries a
    65th all-ones column so row 64 accumulates the softmax denominator.
  - FILLER_N junk-ldweights PE fillers (anchored on pt(i-3) so the
    scheduler cannot hoist them) can pad the PE stream to keep its DVFS
    ramp alive; measured best at 0 under the device's power manager --
    extra PE busy time draws power and invites harder throttling.
  - Drain per pass (overlapped with the next pass's first MM1s): DVE
    copies psO->SBUF fp16, PE transposes into a scores-pool PSUM slot,
    DVE reciprocal of the denominator row + one broadcast tensor_tensor
    mult per head, DMA out.
  - K/V are loaded key-PERMUTED ("(p t) d" instead of "(t p) d") so DMA
    descriptors are contiguous 1-4KB runs per partition instead of 256B;
    the permutation cancels between MM1 and MM2 (scores never leave the
    chip).  Q uses a per-pass permutation unwound by the output DMA's
    "(p j) d" rearrange.
"""

import numpy as np

import concourse.bass as bass
import concourse.bacc as bacc
import concourse.mybir as mybir
import concourse.tile as tile
from concourse.bass_utils import run_bass_kernel_spmd
from concourse.masks import make_identity

F32 = mybir.dt.float32
F16 = mybir.dt.float16

B, H, S, D = 2, 8, 2048, 64
HPC = 2            # heads per core
NCORES = 8
QB = 512           # query block (PSUM bank free-dim limit for fp32)
NQB = S // QB      # 4
KT_TILE = 128      # keys per k-tile (PE contract partition limit)
NKT = S // KT_TILE # 16
NIT = NQB * NKT    # 64 flattened (pass, kt) iterations
SCALE = 1.0 / np.sqrt(np.float32(D))  # 0.125
FILLER_N = 0       # PE-filler ldweights per kt; 0 measured fastest (DVFS power)


def build_nc():
    nc = bacc.Bacc(None)
    Qd = nc.declare_dram_parameter("q", [HPC, S, D], F32, isOutput=False)
    Kd = nc.declare_dram_parameter("k", [S, D], F32, isOutput=False)
    Vd = nc.declare_dram_parameter("v", [S, D], F32, isOutput=False)
    Od = nc.declare_dram_parameter("o", [HPC, S, D], F32, isOutput=True)

    with tile.TileContext(nc) as tc:
        with (
            tc.tile_pool(name="const", bufs=1) as constp,
            tc.tile_pool(name="qk", bufs=1) as qkp,
            tc.tile_pool(name="vt", bufs=1) as vp,
            tc.tile_pool(name="pt", bufs=6) as ptp,
            tc.tile_pool(name="ot", bufs=2) as otp,
            tc.tile_pool(name="outsb", bufs=3) as outp,
            tc.tile_pool(name="rec", bufs=3) as recp,
            tc.tile_pool(name="psS", bufs=3, space="PSUM") as psSp,
            tc.tile_pool(name="psO", bufs=1, space="PSUM") as psOp,
        ):
            # ---- input staging.  The time-critical K and pass-0 Q go over the
            # sync HWDGE queue as fp32 + DVE casts (starts immediately); the
            # identity goes first on gpsimd so PE transposes are never blocked
            # on it; V and the rest of Q use gpsimd cast-DMAs in parallel. ----
            ident16 = constp.tile([128, 128], F16)
            make_identity(nc, ident16[:])

            Kn = qkp.tile([128, NKT, D], F32, name="Kn")
            Kh = qkp.tile([128, NKT, D], F16, name="Kh")
            Kperm = Kd.ap().rearrange("(p t) d -> p t d", p=128)
            nc.sync.dma_start(out=Kn[:, 0:4, :], in_=Kperm[:, 0:4, :])
            Qn0 = qkp.tile([128, HPC, 4, D], F32, name="Qn0")
            qn0_in = [
                Qd.ap()[h, 0:QB, :].rearrange("(p t) d -> p t d", p=128)
                for h in range(HPC)
            ]
            # Spread the staging DMA issues across idle HWDGE queues so the
            # sync queue's serial ~0.6us-per-issue cost doesn't delay K/Q0.
            nc.sync.dma_start(out=Qn0[:, 0, :, :], in_=qn0_in[0])
            nc.scalar.dma_start(out=Qn0[:, 1, :, :], in_=qn0_in[1])
            nc.scalar.dma_start(out=Kn[:, 4:NKT, :], in_=Kperm[:, 4:NKT, :])
            nc.vector.tensor_copy(Kh[:, 0:4, :], Kn[:, 0:4, :])
            Qh = qkp.tile([128, HPC, NKT, D], F16, name="Qh")
            nc.vector.tensor_copy(Qh[:, :, 0:4, :], Qn0[:])
            nc.vector.tensor_copy(Kh[:, 4:NKT, :], Kn[:, 4:NKT, :])

            # Prime the exp table load so the ~2.7us ACT_TABLE_LOAD overlaps
            # the input DMA phase instead of stalling the first real exp.
            dummy = constp.tile([128, 16], F32)
            nc.vector.memset(dummy[:], 0.0)
            nc.scalar.activation(dummy[:], dummy[:], mybir.ActivationFunctionType.Exp)

            # V tiles [128k, kt, 65] fp16, 65th column = 1.0 (denominator).
            Vt = vp.tile([128, NKT, D + 1], F16)
            nc.gpsimd.dma_start(
                out=Vt[:, :, 0:D],
                in_=Vd.ap().rearrange("(p t) d -> p t d", p=128),
            )
            nc.gpsimd.memset(Vt[:, :, D : D + 1], 1.0)
            for h in range(HPC):
                for pp in range(1, NQB):
                    nc.gpsimd.dma_start(
                        out=Qh[:, h, 4 * pp : 4 * pp + 4, :],
                        in_=Qd.ap()[h, pp * QB : (pp + 1) * QB, :].rearrange(
                            "(p t) d -> p t d", p=128
                        ),
                    )

            # KT [128, S]: K^T on partitions 0-63 via PE transposes, then
            # replicated to 64-127 with one SBUF->SBUF DMA.
            # QT [128, S]: head0^T on partitions 0-63, head1^T on 64-127.
            KT = qkp.tile([128, S], F16, name="KT")
            QT = qkp.tile([128, S], F16, name="QT")

            def stage_k(t):
                ts_ = slice(t * 128, (t + 1) * 128)
                psk = psSp.tile([128, 128], F16, name="psk", tag="ps")
                for half in range(2):
                    nc.tensor.transpose(
                        psk[64 * half : 64 * (half + 1), :],
                        Kh[:, t, :],
                        ident16[:],
                        tile_position=(0, 64 * half),
                    )
                nc.vector.tensor_copy(KT[:, ts_], psk[:])

            def stage_q(t):
                ts_ = slice(t * 128, (t + 1) * 128)
                psq = psSp.tile([128, 128], F16, name="psq", tag="ps")
                for h in range(HPC):
                    nc.tensor.transpose(
                        psq[64 * h : 64 * (h + 1), :],
                        Qh[:, h, t, :],
                        ident16[:],
                        tile_position=(0, 64 * h),
                    )
                nc.vector.tensor_copy(QT[:, ts_], psq[:])

            for t in range(4):
                stage_k(t)
            for t in range(4):
                stage_q(t)
            for t in range(4, 6):
                stage_k(t)

            # ---- flattened main loop over (pass, kt) ----
            sc = {}    # i -> score psum tile
            pt = {}    # i -> prob sbuf tile
            pt_hist = {}  # filler anchors
            ps_o = {}  # pass -> psO tile [65, 2, 512]

            def emit_mm1(i):
                if i >= NIT:
                    return
                p, kt = divmod(i, NKT)
                qs = slice(p * QB, (p + 1) * QB)
                ks = slice(kt * KT_TILE, (kt + 1) * KT_TILE)
                ps_s = psSp.tile([128, HPC, QB], F32, name="ps_s", tag="ps")
                for h in range(HPC):
                    nc.tensor.matmul(
                        ps_s[:, h, :],
                        lhsT=KT[64 * h : 64 * (h + 1), ks],
                        rhs=QT[64 * h : 64 * (h + 1), qs],
                        start=True,
                        stop=True,
                    )
                sc[i] = ps_s

            def emit_act(i):
                ptile = ptp.tile([128, HPC, QB], F16, name="ptile")
                nc.scalar.activation(
                    ptile[:],
                    sc.pop(i)[:],
                    mybir.ActivationFunctionType.Exp,
                    scale=float(SCALE),
                )
                pt[i] = ptile

            def emit_mm2(i):
                p, kt = divmod(i, NKT)
                if kt == 0:
                    ps_o[p] = [
                        psOp.tile([D + 1, QB], F32, name="psO", tag=f"psO{h}")
                        for h in range(HPC)
                    ]
                ptile = pt.pop(i)
                for h in range(HPC):
                    nc.tensor.matmul(
                        ps_o[p][h][:],
                        lhsT=Vt[:, kt, :],
                        rhs=ptile[:, h, :],
                        start=(kt == 0),
                        stop=(kt == NKT - 1),
                    )

            def emit_drain(p):
                qs = slice(p * QB, (p + 1) * QB)
                ots = []
                # Both psO->SBUF copies go first (DVE) so the psO WAR clears
                # for the next pass's first MM2 as early as possible.
                for h in range(HPC):
                    ot = otp.tile([D + 1, QB], F16, name="ot")
                    nc.vector.tensor_copy(ot[:], ps_o[p][h][:])
                    ots.append(ot)
                ps_t = psSp.tile([128, HPC, QB // 128, D + 2], F16, name="ps_t", tag="ps")
                for h in range(HPC):
                    for j in range(QB // 128):
                        nc.tensor.transpose(
                            ps_t[:, h, j, 0 : D + 1],
                            ots[h][:, j * 128 : (j + 1) * 128],
                            ident16[0 : D + 1, 0 : D + 1],
                        )
                    rec = recp.tile([128, QB // 128, 1], F32, name="rec")
                    nc.vector.reciprocal(rec[:], ps_t[:, h, :, D : D + 1])
                    outsb = outp.tile([128, QB // 128, D], F32, name="outsb")
                    in0, in1 = bass.broadcast_tensor_aps(ps_t[:, h, :, 0:D], rec[:])
                    nc.vector.tensor_tensor(
                        out=outsb[:], in0=in0, in1=in1, op=mybir.AluOpType.mult
                    )
                    nc.sync.dma_start(
                        out=Od.ap()[h, qs, :].rearrange("(p j) d -> p j d", p=128),
                        in_=outsb[:],
                    )
                del ps_o[p]

            emit_mm1(0)
            emit_mm1(1)
            for i in range(NIT):
                p, kt = divmod(i, NKT)
                # Stage K/Q tiles just-in-time, ahead of the MM1 that first
                # reads them, so the lead-in only pays for what pass 0's
                # start needs and the rest overlaps the pipeline.
                if p == 0 and kt <= 9:
                    stage_k(kt + 6)
                if p < NQB - 1 and kt in (2, 5, 8, 11):
                    stage_q(4 * (p + 1) + (kt - 2) // 3)
                emit_mm1(i + 2)
                # PE filler anchored 3 tiles back: junk weight loads that keep
                # the tensor engine's DVFS ramp alive (2.4GHz needs ~3us of
                # continuous busy) during the ACT->MM2 waits.  The pt(i-3)
                # read pins them into the pipeline (no hoisting to the head of
                # the program) but is always satisfied when the PE gets here,
                # so they never gate real work (pt bufs=6 keeps the ACT WAR
                # on those slots out of the way too).
                if i >= 3:
                    for f in range(FILLER_N):
                        nc.tensor.ldweights(pt3[0:64, f, 0:128])
                emit_act(i)
                if i >= 2:
                    pt3 = pt_hist[i - 2]
                pt_hist[i] = pt[i]
                emit_mm2(i)
                pt_hist.pop(i - 4, None)
                if kt == NKT - 1:
                    emit_drain(p)
    nc.compile()
    return nc


_CACHED = {}


def _get_nc():
    if "nc" not in _CACHED:
        _CACHED["nc"] = build_nc()
    return _CACHED["nc"]


def _shard(Q, K, V):
    in_maps = []
    for c in range(NCORES):
        b = c // 4
        h0 = (c % 4) * HPC
        in_maps.append(
            {
                "q": np.ascontiguousarray(np.asarray(Q, np.float32)[b, h0 : h0 + HPC]),
                "k": np.ascontiguousarray(np.asarray(K, np.float32)[b, 0]),
                "v": np.ascontiguousarray(np.asarray(V, np.float32)[b, 0]),
            }
        )
    return in_maps


def kernel(Q, K, V, trace=False):
    nc = _get_nc()
    res = run_bass_kernel_spmd(nc, _shard(Q, K, V), list(range(NCORES)), trace=trace)
    _CACHED["last_result"] = res
    O = np.empty((B, H, S, D), np.float32)
    for c, r in enumerate(res.results):
        b = c // 4
        h0 = (c % 4) * HPC
        O[b, h0 : h0 + HPC] = r["o"]
    return O


# revision 31
# speedup vs baseline: 1.0049x; 1.0049x over previous
"""MQA attention kernel for Trainium2, 8-core SPMD.

Problem: Q [2, 8, 2048, 64] fp32, K/V [2, 1, 2048, 64] fp32 (shared head).
out[b,h,q,:] = softmax(Q[b,h,q,:] @ K[b,0]^T / 8) @ V[b,0].

Sharding: 16 (b,h) pairs over 8 cores -> core c handles b = c//4,
heads 2*(c%4), 2*(c%4)+1 (both heads share one K/V slice).

Per-core design (matmuls fp16, accumulation fp32 in PSUM).  The kernel is
bound by the scalar engine's exp throughput (1 elem/cycle/lane @ 1.2GHz:
~1.0-1.1us per [128, 2x512] score tile, 64 tiles), so everything else is
arranged to keep the ACT stream saturated:

  - Staging: the time-critical K and pass-0 Q arrive over the sync HWDGE
    queue as fp32 with DVE casts; V and the remaining Q use gpsimd cast
    DMAs in parallel.  PE transposes (fp16) build K^T/Q^T incrementally --
    the lead-in stages only what pass 0's first iterations need, the rest
    is staged just-in-time inside the main loop, 2 tiles ahead of the MM1
    that first reads it.
  - Main loop: one flat stream over i = (pass, kt); MM1 for i+2 is emitted
    ahead of MM2(i) (scores triple-buffered in PSUM) so the ACT->MM2
    dependence never idles the PE; the two heads' MM1s run concurrently on
    different PE row-quadrants (contract=64).
  - ACT: exp of the [128, 2, 512] fp32 score tile with the 1/8 scale
    folded into the activation's free affine, writing fp16 P^T directly.
  - MM2: out^T[h][65, 512] += V_aug[kt]^T @ P^T[kt, h]; V_aug carries a
    65th all-ones column so row 64 accumulates the softmax denominator.
  - FILLER_N junk-ldweights PE fillers (anchored on pt(i-3) so the
    scheduler cannot hoist them) can pad the PE stream to keep its DVFS
    ramp alive; measured best at 0 under the device's power manager --
    extra PE busy time draws power and invites harder throttling.
  - Drain per pass (overlapped with the next pass's first MM1s): DVE
    copies psO->SBUF fp16, PE transposes into a scores-pool PSUM slot,
    DVE reciprocal of the denominator row + one broadcast tensor_tensor
    mult per head, DMA out.
  - K/V are loaded key-PERMUTED ("(p t) d" instead of "(t p) d") so DMA
    descriptors are contiguous 1-4KB runs per partition instead of 256B;
    the permutation cancels between MM1 and MM2 (scores never leave the
    chip).  Q uses a per-pass permutation unwound by the output DMA's
    "(p j) d" rearrange.
"""

import numpy as np

import concourse.bass as bass
import concourse.bacc as bacc
import concourse.mybir as mybir
import concourse.tile as tile
from concourse.bass_utils import run_bass_kernel_spmd
from concourse.masks import make_identity

F32 = mybir.dt.float32
F16 = mybir.dt.float16

B, H, S, D = 2, 8, 2048, 64
HPC = 2            # heads per core
NCORES = 8
QB = 512           # query block (PSUM bank free-dim limit for fp32)
NQB = S // QB      # 4
KT_TILE = 128      # keys per k-tile (PE contract partition limit)
NKT = S // KT_TILE # 16
NIT = NQB * NKT    # 64 flattened (pass, kt) iterations
SCALE = 1.0 / np.sqrt(np.float32(D))  # 0.125
FILLER_N = 0       # PE-filler ldweights per kt; 0 measured fastest (DVFS power)


def build_nc():
    nc = bacc.Bacc(None)
    Qd = nc.declare_dram_parameter("q", [HPC, S, D], F32, isOutput=False)
    Kd = nc.declare_dram_parameter("k", [S, D], F32, isOutput=False)
    Vd = nc.declare_dram_parameter("v", [S, D], F32, isOutput=False)
    Od = nc.declare_dram_parameter("o", [HPC, S, D], F32, isOutput=True)

    with tile.TileContext(nc) as tc:
        with (
            tc.tile_pool(name="const", bufs=1) as constp,
            tc.tile_pool(name="qk", bufs=1) as qkp,
            tc.tile_pool(name="vt", bufs=1) as vp,
            tc.tile_pool(name="pt", bufs=6) as ptp,
            tc.tile_pool(name="ot", bufs=2) as otp,
            tc.tile_pool(name="outsb", bufs=3) as outp,
            tc.tile_pool(name="rec", bufs=3) as recp,
            tc.tile_pool(name="psS", bufs=3, space="PSUM") as psSp,
            tc.tile_pool(name="psO", bufs=1, space="PSUM") as psOp,
        ):
            # ---- input staging.  The time-critical K and pass-0 Q go over the
            # sync HWDGE queue as fp32 + DVE casts (starts immediately); the
            # identity goes first on gpsimd so PE transposes are never blocked
            # on it; V and the rest of Q use gpsimd cast-DMAs in parallel. ----
            ident16 = constp.tile([128, 128], F16)
            make_identity(nc, ident16[:])

            Kn = qkp.tile([128, NKT, D], F32, name="Kn")
            Kh = qkp.tile([128, NKT, D], F16, name="Kh")
            Kperm = Kd.ap().rearrange("(p t) d -> p t d", p=128)
            nc.sync.dma_start(out=Kn[:, 0:4, :], in_=Kperm[:, 0:4, :])
            Qn0 = qkp.tile([128, HPC, 4, D], F32, name="Qn0")
            qn0_in = [
                Qd.ap()[h, 0:QB, :].rearrange("(p t) d -> p t d", p=128)
                for h in range(HPC)
            ]
            # Spread the staging DMA issues across idle HWDGE queues so the
            # sync queue's serial ~0.6us-per-issue cost doesn't delay K/Q0.
            nc.sync.dma_start(out=Qn0[:, 0, :, :], in_=qn0_in[0])
            nc.scalar.dma_start(out=Qn0[:, 1, :, :], in_=qn0_in[1])
            nc.scalar.dma_start(out=Kn[:, 4:NKT, :], in_=Kperm[:, 4:NKT, :])
            nc.vector.tensor_copy(Kh[:, 0:4, :], Kn[:, 0:4, :])
            Qh = qkp.tile([128, HPC, NKT, D], F16, name="Qh")
            nc.vector.tensor_copy(Qh[:, :, 0:4, :], Qn0[:])
            nc.vector.tensor_copy(Kh[:, 4:NKT, :], Kn[:, 4:NKT, :])

            # Prime the exp table load so the ~2.7us ACT_TABLE_LOAD overlaps
            # the input DMA phase instead of stalling the first real exp.
            dummy = constp.tile([128, 16], F32)
            nc.vector.memset(dummy[:], 0.0)
            nc.scalar.activation(dummy[:], dummy[:], mybir.ActivationFunctionType.Exp)

            # V tiles [128k, kt, 65] fp16, 65th column = 1.0 (denominator).
            Vt = vp.tile([128, NKT, D + 1], F16)
            nc.gpsimd.dma_start(
                out=Vt[:, :, 0:D],
                in_=Vd.ap().rearrange("(p t) d -> p t d", p=128),
            )
            nc.gpsimd.memset(Vt[:, :, D : D + 1], 1.0)
            for h in range(HPC):
                for pp in range(1, NQB):
                    nc.gpsimd.dma_start(
                        out=Qh[:, h, 4 * pp : 4 * pp + 4, :],
                        in_=Qd.ap()[h, pp * QB : (pp + 1) * QB, :].rearrange(
                            "(p t) d -> p t d", p=128
                        ),
                    )

            # KT [128, S]: K^T on partitions 0-63 via PE transposes, then
            # replicated to 64-127 with one SBUF->SBUF DMA.
            # QT [128, S]: head0^T on partitions 0-63, head1^T on 64-127.
            KT = qkp.tile([128, S], F16, name="KT")
            QT = qkp.tile([128, S], F16, name="QT")

            def stage_k(t):
                ts_ = slice(t * 128, (t + 1) * 128)
                psk = psSp.tile([128, 128], F16, name="psk", tag="ps")
                for half in range(2):
                    nc.tensor.transpose(
                        psk[64 * half : 64 * (half + 1), :],
                        Kh[:, t, :],
                        ident16[:],
                        tile_position=(0, 64 * half),
                    )
                nc.vector.tensor_copy(KT[:, ts_], psk[:])

            def stage_q(t):
                ts_ = slice(t * 128, (t + 1) * 128)
                psq = psSp.tile([128, 128], F16, name="psq", tag="ps")
                for h in range(HPC):
                    nc.tensor.transpose(
                        psq[64 * h : 64 * (h + 1), :],
                        Qh[:, h, t, :],
                        ident16[:],
                        tile_position=(0, 64 * h),
                    )
                nc.vector.tensor_copy(QT[:, ts_], psq[:])

            for t in range(4):
                stage_k(t)
            for t in range(4):
                stage_q(t)

            # ---- flattened main loop over (pass, kt) ----
            sc = {}    # i -> score psum tile
            pt = {}    # i -> prob sbuf tile
            pt_hist = {}  # filler anchors
            ps_o = {}  # pass -> psO tile [65, 2, 512]

            def emit_mm1(i):
                if i >= NIT:
                    return
                p, kt = divmod(i, NKT)
                qs = slice(p * QB, (p + 1) * QB)
                ks = slice(kt * KT_TILE, (kt + 1) * KT_TILE)
                ps_s = psSp.tile([128, HPC, QB], F32, name="ps_s", tag="ps")
                for h in range(HPC):
                    nc.tensor.matmul(
                        ps_s[:, h, :],
                        lhsT=KT[64 * h : 64 * (h + 1), ks],
                        rhs=QT[64 * h : 64 * (h + 1), qs],
                        start=True,
                        stop=True,
                    )
                sc[i] = ps_s

            def emit_act(i):
                ptile = ptp.tile([128, HPC, QB], F16, name="ptile")
                nc.scalar.activation(
                    ptile[:],
                    sc.pop(i)[:],
                    mybir.ActivationFunctionType.Exp,
                    scale=float(SCALE),
                )
                pt[i] = ptile

            def emit_mm2(i):
                p, kt = divmod(i, NKT)
                if kt == 0:
                    ps_o[p] = [
                        psOp.tile([D + 1, QB], F32, name="psO", tag=f"psO{h}")
                        for h in range(HPC)
                    ]
                ptile = pt.pop(i)
                for h in range(HPC):
                    nc.tensor.matmul(
                        ps_o[p][h][:],
                        lhsT=Vt[:, kt, :],
                        rhs=ptile[:, h, :],
                        start=(kt == 0),
                        stop=(kt == NKT - 1),
                    )

            def emit_drain(p):
                qs = slice(p * QB, (p + 1) * QB)
                ots = []
                # Both psO->SBUF copies go first (DVE) so the psO WAR clears
                # for the next pass's first MM2 as early as possible.
                for h in range(HPC):
                    ot = otp.tile([D + 1, QB], F16, name="ot")
                    nc.vector.tensor_copy(ot[:], ps_o[p][h][:])
                    ots.append(ot)
                ps_t = psSp.tile([128, HPC, QB // 128, D + 2], F16, name="ps_t", tag="ps")
                for h in range(HPC):
                    for j in range(QB // 128):
                        nc.tensor.transpose(
                            ps_t[:, h, j, 0 : D + 1],
                            ots[h][:, j * 128 : (j + 1) * 128],
                            ident16[0 : D + 1, 0 : D + 1],
                        )
                    rec = recp.tile([128, QB // 128, 1], F32, name="rec")
                    nc.vector.reciprocal(rec[:], ps_t[:, h, :, D : D + 1])
                    outsb = outp.tile([128, QB // 128, D], F32, name="outsb")
                    in0, in1 = bass.broadcast_tensor_aps(ps_t[:, h, :, 0:D], rec[:])
                    nc.vector.tensor_tensor(
                        out=outsb[:], in0=in0, in1=in1, op=mybir.AluOpType.mult
                    )
                    nc.sync.dma_start(
                        out=Od.ap()[h, qs, :].rearrange("(p j) d -> p j d", p=128),
                        in_=outsb[:],
                    )
                del ps_o[p]

            emit_mm1(0)
            emit_mm1(1)
            for i in range(NIT):
                p, kt = divmod(i, NKT)
                # Stage K/Q tiles just-in-time, ahead of the MM1 that first
                # reads them, so the lead-in only pays for what pass 0's
                # start needs and the rest overlaps the pipeline.
                if p == 0 and kt <= 11:
                    stage_k(kt + 4)
                if p < NQB - 1 and kt in (2, 5, 8, 11):
                    stage_q(4 * (p + 1) + (kt - 2) // 3)
                emit_mm1(i + 2)
                # PE filler anchored 3 tiles back: junk weight loads that keep
                # the tensor engine's DVFS ramp alive (2.4GHz needs ~3us of
                # continuous busy) during the ACT->MM2 waits.  The pt(i-3)
                # read pins them into the pipeline (no hoisting to the head of
                # the program) but is always satisfied when the PE gets here,
                # so they never gate real work (pt bufs=6 keeps the ACT WAR
                # on those slots out of the way too).
                if i >= 3:
                    for f in range(FILLER_N):
                        nc.tensor.ldweights(pt3[0:64, f, 0:128])
                emit_act(i)
                if i >= 2:
                    pt3 = pt_hist[i - 2]
                pt_hist[i] = pt[i]
                emit_mm2(i)
                pt_hist.pop(i - 4, None)
                if kt == NKT - 1:
                    emit_drain(p)
    nc.compile()
    return nc


_CACHED = {}


def _get_nc():
    if "nc" not in _CACHED:
        _CACHED["nc"] = build_nc()
    return _CACHED["nc"]


def _shard(Q, K, V):
    in_maps = []
    for c in range(NCORES):
        b = c // 4
        h0 = (c % 4) * HPC
        in_maps.append(
            {
                "q": np.ascontiguousarray(np.asarray(Q, np.float32)[b, h0 : h0 + HPC]),
                "k": np.ascontiguousarray(np.asarray(K, np.float32)[b, 0]),
                "v": np.ascontiguousarray(np.asarray(V, np.float32)[b, 0]),
            }
        )
    return in_maps


def kernel(Q, K, V, trace=False):
    nc = _get_nc()
    res = run_bass_kernel_spmd(nc, _shard(Q, K, V), list(range(NCORES)), trace=trace)
    _CACHED["last_result"] = res
    O = np.empty((B, H, S, D), np.float32)
    for c, r in enumerate(res.results):
        b = c // 4
        h0 = (c % 4) * HPC
        O[b, h0 : h0 + HPC] = r["o"]
    return O
